# revision 78
# baseline (speedup 1.0000x reference)
"""Multi-head attention Trainium2 Bass kernel, sharded over 8 NeuronCores.

Problem: B=4, S=2048, D=1024, H=16 heads (DK=64), fp32, random 0/1 mask.

Sharding (data-parallel batch x tensor-parallel heads):
  core c handles batch b = c // 2, head-group hg = c % 2 (8 heads = 512 dims).
  Host sums the two head-group partials per batch and adds the fused output
  bias (bo + wo @ bv, exact because softmax weights sum to 1).

v3 dataflow (v2 + engine rebalance and a sliding exp-prefetch cursor):
  - All inputs stream bf16 (x, weights, mask, output) — halves DMA traffic;
    error stays ~2.4e-3, well inside the 2e-2 gate.
  - Scores come out transposed S^T = [k, q] per head-pair (two heads packed
    into the 128 PE partitions as 64-row halves). exp() on the scalar engine
    (PSUM -> SBUF bf16) — ACT does ONLY exp (~266us, its floor); every other
    PSUM drain lives on DVE (GPSIMD cannot touch PSUM), mask multiplies are
    split DVE (2x bf16 mode) / Pool.
  - attn@V is FLIPPED: stationary = masked-exp tile E^T [128 k, 128 q],
    moving = V tile [128 k, 65] (64 dims + ones column) -> out C [128 q, 65]
    accumulated over the 16 k tiles in PSUM; PSUM column 64 is the softmax
    denominator Z (ones-column trick). reciprocal + per-partition
    tensor_scalar normalize, then a bf16 PE-transpose (1 cycle/row) feeds
    the wo projection at full PE packing.
  - A sliding cursor emits score+exp tasks ahead of PV consumption, bounded
    by the e-tile pool (CAP=34) and data readiness; it may skip gated tasks
    out of order and crosses into the next q-chunk from j1 (qproj/masks for
    qc+1 are hoisted into j1). Consumption drains every seasoned cached
    tile per slot (bounded by the j0 vproj stream) so the pool recycles
    during kproj/vproj bursts. This keeps ACT's exp stream fed through the
    PE-bound first chunk, where all K/V projection deadlines live.
  - Startup: preamble DMAs are consolidated and spread across the SP /
    Pool / ACT queues, and ~34 warmup matmuls ramp the PE p-state to full
    clock before the first kproj (213 vs 788 ns per 512-row group).
  - Tail: 6 output chunks get jj0-2 partial matmuls before the last
    normalize; the final transpose and last two output chunks run on
    recycled psPV/psS banks; tail copies alternate DVE/ACT.
  Engines end ~88% (PE 280us busy) / ~86% (ACT 274us) of the 318us total.
"""
import numpy as np

import concourse.bass as bass
import concourse.mybir as mybir
import concourse.tile as tile
from concourse import bacc
from concourse.masks import make_identity

B, S, D, H = 4, 2048, 1024, 16
DK = D // H          # 64
NCORES = 8
HG = 2               # head groups (tensor-parallel factor per batch)
HPG = H // HG        # 8 heads per core
DH = D // HG         # 512 head dims per core
QCN = 4              # q chunks
QCS = S // QCN       # 512
KT = S // 128        # 16 k tiles
DT = D // 128        # 8 contraction tiles for projections
F32 = mybir.dt.float32
F32R = mybir.dt.float32r
BF16 = mybir.dt.bfloat16
NSPILL = 16          # DRAM spill slots for prefetched exp tiles
MULT = mybir.AluOpType.mult
ADD = mybir.AluOpType.add
EXP = mybir.ActivationFunctionType.Exp
IDENT = mybir.ActivationFunctionType.Identity


def build_nc():
    nc = bacc.Bacc(None)
    xqT = nc.declare_dram_parameter("xqT", [D, S], BF16, isOutput=False)
    xkT = nc.declare_dram_parameter("xkT", [D, S], BF16, isOutput=False)
    xvT = nc.declare_dram_parameter("xvT", [D, S], BF16, isOutput=False)
    maskT = nc.declare_dram_parameter("maskT", [S, S], BF16, isOutput=False)
    wqT = nc.declare_dram_parameter("wqT", [D, DH], BF16, isOutput=False)
    wkT = nc.declare_dram_parameter("wkT", [D, DH], BF16, isOutput=False)
    wvT = nc.declare_dram_parameter("wvT", [D, DH], BF16, isOutput=False)
    woT = nc.declare_dram_parameter("woT", [DH, D], BF16, isOutput=False)
    bq2 = nc.declare_dram_parameter("bq2", [128, DH // 128], F32, isOutput=False)
    bk2 = nc.declare_dram_parameter("bk2", [128, DH // 128], F32, isOutput=False)
    esp = nc.declare_dram_parameter("esp", [NSPILL * 128, 2 * QCS], BF16,
                                    isOutput=False)
    outT = nc.declare_dram_parameter("outT", [D, S], BF16, isOutput=True)

    with tile.TileContext(nc) as tc:
        with (
            tc.tile_pool(name="persist", bufs=1) as pp,
            tc.tile_pool(name="xp", bufs=2) as xp,
            tc.tile_pool(name="qtp", bufs=2) as qtp,
            tc.tile_pool(name="maskp", bufs=4) as maskp,
            tc.tile_pool(name="ep", bufs=36) as ep,
            tc.tile_pool(name="nrmp", bufs=2) as nrmp,
            tc.tile_pool(name="ctp", bufs=8) as ctp,
            tc.tile_pool(name="op", bufs=4) as op_,
            tc.tile_pool(name="zp", bufs=2) as zp,
            tc.tile_pool(name="psS", bufs=2, space="PSUM") as psS,
            tc.tile_pool(name="psPV", bufs=1, space="PSUM") as psPV,
            tc.tile_pool(name="psC", bufs=2, space="PSUM") as psC,
        ):
            # ---- persistent tiles ----
            kt_full = pp.tile([128, 4, S], BF16, tag="kt", name="kt_full")
            v_full = pp.tile([128, KT, HPG * 65], BF16, tag="v", name="v_full")
            wq_full = pp.tile([128, DT, DH], BF16, tag="wq", name="wq_full")
            wk_full = pp.tile([128, DT, DH], BF16, tag="wk", name="wk_full")
            wv_full = pp.tile([128, DT, DH], BF16, tag="wv", name="wv_full")
            wo_full = pp.tile([128, 4, D], BF16, tag="wo", name="wo_full")
            bias_sb = pp.tile([128, 8], F32, tag="bias", name="bias_sb")
            ident = pp.tile([128, 128], BF16, tag="ident", name="ident")
            make_identity(nc, ident)
            bq_sb = bias_sb[:, 0:4]   # bq / 8, one col per dh tile
            bk_sb = bias_sb[:, 4:8]

            # ---- DMA loaders ----
            def load_xk(c):
                t = xp.tile([128, DT, QCS], BF16, tag="x", name="xk_t")
                xs = xkT[:, c * QCS:(c + 1) * QCS].rearrange(
                    "(t p) s -> p t s", p=128)
                nc.sync.dma_start(t[:, 0:4, :], xs[:, 0:4, :])
                nc.gpsimd.dma_start(t[:, 4:8, :], xs[:, 4:8, :])
                return t

            def load_xq(qc, interleave_with=None):
                t = xp.tile([128, DT, QCS], BF16, tag="x", name="xq_t")
                for ch in range(2):
                    xs = xqT[ch * (D // 2):(ch + 1) * (D // 2),
                             qc * QCS:(qc + 1) * QCS].rearrange(
                        "(t p) s -> p t s", p=128)
                    (nc.sync if ch == 0 else nc.gpsimd).dma_start(
                        t[:, ch * 4:(ch + 1) * 4, :], xs)
                    if ch == 0 and interleave_with is not None:
                        interleave_with()
                return t

            def load_xv(c):
                t = xp.tile([128, DT, QCS], BF16, tag="x", name="xv_t")
                xs = xvT[:, c * QCS:(c + 1) * QCS].rearrange(
                    "(t p) s -> p t s", p=128)
                nc.gpsimd.dma_start(t[:, 0:4, :], xs[:, 0:4, :])
                nc.sync.dma_start(t[:, 4:8, :], xs[:, 4:8, :])
                return t

            def load_mask_half(qc, half, engs=None):
                t = maskp.tile([128, KT // 2, QCS], BF16, tag="m",
                               name="mask_t")
                hm = KT // 2
                ms = maskT[half * (S // 2):(half + 1) * (S // 2),
                           qc * QCS:(qc + 1) * QCS].rearrange(
                    "(t p) s -> p t s", p=128)
                e0, e1 = engs if engs else (nc.sync, nc.gpsimd)
                e0.dma_start(t[:, 0:hm // 2, :], ms[:, 0:hm // 2, :])
                e1.dma_start(t[:, hm // 2:hm, :], ms[:, hm // 2:hm, :])
                return t

            # ---- projection groups ----
            def kproj(c, d, xk_t):
                """K^T rows d*128.. for the k columns of chunk c."""
                ps = psC.tile([128, QCS], F32, tag="ps", name="ps_k")
                for i in range(DT):
                    nc.tensor.matmul(
                        ps[:], wk_full[:, i, d * 128:(d + 1) * 128],
                        xk_t[:, i, :], start=(i == 0), stop=(i == DT - 1))
                nc.vector.tensor_scalar_add(
                    kt_full[:, d, c * QCS:(c + 1) * QCS], ps[:],
                    bk_sb[:, d:d + 1])

            def qproj(d, qt_t, xq_t, on_act=False):
                ps = psC.tile([128, QCS], F32, tag="ps", name="ps_q")
                for i in range(DT):
                    nc.tensor.matmul(
                        ps[:], wq_full[:, i, d * 128:(d + 1) * 128],
                        xq_t[:, i, :], start=(i == 0), stop=(i == DT - 1))
                nc.vector.tensor_scalar(
                    qt_t[:, d, :], ps[:], 0.125, bq_sb[:, d:d + 1],
                    MULT, ADD)

            def vproj(st, xv_t):
                ps = psC.tile([128, QCS], F32, tag="ps", name="ps_v")
                sub = st % 4
                for i in range(DT):
                    nc.tensor.matmul(
                        ps[:], xv_t[:, i, sub * 128:(sub + 1) * 128],
                        wv_full[:, i, :], start=(i == 0), stop=(i == DT - 1))
                vdst = v_full[:, st, :].rearrange(
                    "p (h c) -> p h c", h=HPG)[:, :, 0:64]
                nc.vector.tensor_copy(
                    vdst, ps[:].rearrange("p (h c) -> p h c", h=HPG))

            # ---- attention pieces ----
            def scores_exp_mask(j, kt, qt_t, mask_t):
                mh = mask_t[kt // 8]
                sps = psS.tile([128, 2, QCS], F32, tag="sps", name="sps")
                for hh in range(2):
                    prow = hh * 64
                    nc.tensor.matmul(
                        sps[:, hh, :],
                        kt_full[prow:prow + 64, j, kt * 128:(kt + 1) * 128],
                        qt_t[prow:prow + 64, j, :],
                        start=True, stop=True)
                e_t = ep.tile([128, 2, QCS], BF16, tag="e", name="e_t")
                nc.scalar.activation(e_t[:], sps[:], EXP)
                meng = nc.gpsimd if (kt % 2 == 1 or kt < 2) else nc.vector
                for hh in range(2):
                    meng.tensor_mul(
                        e_t[:, hh, :], e_t[:, hh, :], mh[:, kt % 8, :])
                return e_t

            def pv_accum(j, kt, e_t, pv):
                # PSUM start=True clears has_written for the WHOLE bank, so
                # only the bank's first matmul (kt0, qt0) may set it. Later
                # qt groups' first writes land on cleared bits and overwrite
                # per-element; stop closes the bank after the last group.
                for qt in range(4):
                    for hh in range(2):
                        h = 2 * j + hh
                        nc.tensor.matmul(
                            pv[hh][:, qt, 0:65],
                            e_t[:, hh, qt * 128:(qt + 1) * 128],
                            v_full[:, kt, h * 65:(h + 1) * 65],
                            start=(kt == 0 and qt == 0),
                            stop=(kt == KT - 1 and qt == 3),
                            skip_group_check=True)

            def normalize_transpose(j, pv, ct_cur, tail=False):
                z_t = zp.tile([128, 2, 8], F32, tag="z", name="z_t")
                for hh in range(2):
                    nc.vector.tensor_copy(
                        z_t[:, 0, hh * 4:(hh + 1) * 4],
                        pv[hh][:, :, 64:65].rearrange("p a b -> p (a b)"))
                nc.vector.reciprocal_approx_fast(
                    out=z_t[:, 1, :], in_=z_t[:, 0, :])
                nrm_t = nrmp.tile([128, 4, 2, DK], BF16, tag="n",
                                  name="nrm_t")
                for qt in range(4):
                    for hh in range(2):
                        if tail and hh == 1:
                            # scalar engine is idle at the tail; split the
                            # normalize across ACT + DVE
                            nc.scalar.activation(
                                nrm_t[:, qt, hh, :], pv[hh][:, qt, 0:64],
                                IDENT,
                                scale=z_t[:, 1, hh * 4 + qt:hh * 4 + qt + 1])
                        else:
                            nc.vector.tensor_scalar_mul(
                                nrm_t[:, qt, hh, :], pv[hh][:, qt, 0:64],
                                z_t[:, 1, hh * 4 + qt:hh * 4 + qt + 1])
                return nrm_t

            def transpose_ct(nrm_t, ct_cur, j):
                # PE transpose via identity (the DMA xbar transpose corrupts
                # under concurrent DMA traffic), 4 qt blocks into one bank.
                # Emitted one slot into the NEXT block so the PE queue never
                # stalls on the normalize chain.
                ctps = psC.tile([128, QCS], BF16, tag="ps", name="ctps")
                ctv = ctps[:].rearrange("p (a b) -> p a b", a=4)
                for qt in range(4):
                    nc.tensor.matmul(
                        ctv[:, qt, :], nrm_t[:, qt, :, :], ident[:],
                        is_transpose=True, start=(qt == 0), stop=(qt == 3),
                        skip_group_check=True)
                ct_cur[j] = ctp.tile([128, 4, 128], BF16, tag="ct",
                                     name="ct_t")
                nc.vector.tensor_copy(ct_cur[j][:], ctv)

            def transpose_ct_tail(nrm_t, ct_cur, j):
                tr_ps = psPV.tile([128, 4, 128], BF16, tag="pv0",
                                  name="tr_ps")
                for qt in range(4):
                    nc.tensor.matmul(
                        tr_ps[:, qt, :], nrm_t[:, qt, :, :], ident[:],
                        is_transpose=True, start=(qt == 0), stop=(qt == 3),
                        skip_group_check=True)
                ct_cur[j] = ctp.tile([128, 4, 128], BF16, tag="ct",
                                     name="ct_t")
                nc.scalar.activation(ct_cur[j][:], tr_ps[:], IDENT)

            def outproj(qc_prev, oc, ct_prev, po=None, on_act=False):
                if po is None:
                    po = psC.tile([128, QCS], F32, tag="ps", name="po")[:]
                for jj in range(4):
                    nc.tensor.matmul(
                        po, wo_full[:, jj, oc * 128:(oc + 1) * 128],
                        ct_prev[jj][:], start=(jj == 0), stop=(jj == 3))
                o_t = op_.tile([128, QCS], BF16, tag="o", name="o_t")
                nc.vector.tensor_copy(o_t[:], po)
                (nc.sync if oc % 2 == 0 else nc.gpsimd).dma_start(
                    outT[oc * 128:(oc + 1) * 128,
                         qc_prev * QCS:(qc_prev + 1) * QCS], o_t[:])

            # ================= emission =================
            # The lead is bound by the serial DMA device. Deliver only what
            # the FIRST scores need (wk/wq dtile-0 columns + xk0 + xq), run
            # kproj(c0,d0) + qproj(d0) + the first scores, then stream the
            # rest (wk/wq d1-3 columns, mask, wv, xv0) behind them.
            e_cache = {}        # (qc, j, kt) -> e tile emitted ahead
            # ---- preamble: startup DMAs spread over all 5 queues ----
            # K-side on sync/gpsimd, Q-side on the idle scalar/vector
            # queues, consolidated so each queue carries ~2 descriptors
            # before the first kproj/qproj inputs are complete.
            xk_cur = xp.tile([128, DT, QCS], BF16, tag="x", name="xk_t")
            xq_cur = xp.tile([128, DT, QCS], BF16, tag="x", name="xq_t")
            xk0 = xkT[:, 0:QCS].rearrange("(t p) s -> p t s", p=128)
            xq0 = xqT[:, 0:QCS].rearrange("(t p) s -> p t s", p=128)
            nc.scalar.dma_start(xq_cur[:, 0:4, :], xq0[:, 0:4, :])
            nc.sync.dma_start(
                wk_full[:, :, 0:128],
                wkT[:, 0:128].rearrange("(t p) c -> p t c", p=128))
            nc.gpsimd.dma_start(xq_cur[:, 4:8, :], xq0[:, 4:8, :])
            nc.scalar.dma_start(
                wq_full[:, :, 0:128],
                wqT[:, 0:128].rearrange("(t p) c -> p t c", p=128))
            nc.sync.dma_start(xk_cur[:, 0:4, :], xk0[:, 0:4, :])
            nc.gpsimd.dma_start(xk_cur[:, 4:8, :], xk0[:, 4:8, :])
            nc.scalar.dma_start(bq_sb[:, :], bq2[:])
            nc.scalar.dma_start(bk_sb[:, :], bk2[:])
            # d1-3 weight columns next: qproj/kproj d1-3 gate the cursor's
            # j1+ score emission, so they outrank the mask halves.
            nc.sync.dma_start(
                wk_full[:, 0:4, 128:DH],
                wkT[0:512, 128:DH].rearrange("(t p) c -> p t c", p=128))
            nc.gpsimd.dma_start(
                wq_full[:, :, 128:DH],
                wqT[:, 128:DH].rearrange("(t p) c -> p t c", p=128))
            nc.scalar.dma_start(
                wk_full[:, 4:8, 128:DH],
                wkT[512:D, 128:DH].rearrange("(t p) c -> p t c", p=128))
            mask_cur = [load_mask_half(0, 0), None]
            ones = v_full[:].rearrange(
                "p t (h c) -> p t h c", h=HPG)[:, :, :, 64:65]
            nc.gpsimd.memset(ones, 1.0)

            # PE p-state warmup: ~34 tiny self-matmuls ramp the tensor
            # engine to full clock while the first DMAs land, so kproj
            # runs at 213 ns/group-row instead of 788.
            warm = pp.tile([128, 128], BF16, tag="warm", name="warm")
            nc.vector.memset(warm, 0.0)
            pswarm = psC.tile([128, QCS], F32, tag="ps", name="pswarm")
            for _ in range(34):
                nc.tensor.matmul(pswarm[:, 0:128], warm[:], warm[:],
                                 start=True, stop=True)

            kproj(0, 0, xk_cur)
            qt_cur = qtp.tile([128, 4, QCS], BF16, tag="qt", name="qt_t")
            qproj(0, qt_cur, xq_cur)
            e_cache[(0, 0, 0)] = scores_exp_mask(0, 0, qt_cur, mask_cur)
            e_cache[(0, 0, 1)] = scores_exp_mask(0, 1, qt_cur, mask_cur)
            kproj(0, 1, xk_cur)
            e_cache[(0, 0, 2)] = scores_exp_mask(0, 2, qt_cur, mask_cur)
            kproj(0, 2, xk_cur)
            e_cache[(0, 0, 3)] = scores_exp_mask(0, 3, qt_cur, mask_cur)
            kproj(0, 3, xk_cur)
            for d in range(1, 4):
                qproj(d, qt_cur, xq_cur)

            mask_cur[1] = load_mask_half(0, 1)

            # wv + first xv chunk for the V bursts inside (qc0, j0)
            nc.sync.dma_start(
                wv_full[:, 0:4, :],
                wvT[0:512, :].rearrange("(t p) c -> p t c", p=128))
            nc.gpsimd.dma_start(
                wv_full[:, 4:8, :],
                wvT[512:D, :].rearrange("(t p) c -> p t c", p=128))
            xv_cur = load_xv(0)

            tail_sps = None
            tail_po = {}
            pend_tr = None
            mask_next = None
            qt_next = None
            xq_next = None
            xk_next = None
            ct_prev = None      # ct tile of previous qc (for outproj)

            # --- sliding score-emission cursor -------------------------
            # All 256 (qc, j, kt) score tasks in consumption order. pump()
            # emits up to `maxn` of them ahead of consumption, bounded by
            # the e-tile pool (CAP) and data availability: same-chunk tasks
            # gate on the kproj burst for their k-chunk (qc0 only);
            # next-chunk tasks gate on qproj(d=j) / mask halves in j3.
            # Emission may SKIP over a gated task (out of order) so one
            # blocked k-chunk never starves the exp stream; `emitted`
            # tracks holes the consumption loop fills on demand.
            tasks = [(tqc, tj, tkt) for tqc in range(QCN)
                     for tj in range(4) for tkt in range(KT)]
            emitted = set()
            emit_seq = {}
            seq = [0]
            cursor = {"i": 0, "qc": 0, "j": 0, "kt": 0}
            CAP = 31
            LOOKAHEAD = 96
            KREADY = {0: -1, 1: 3, 2: 7, 3: 11}   # qc0 kproj burst slots

            def emit_task(t, qt_t, mask_t):
                emitted.add(t)
                emit_seq[t] = seq[0]
                seq[0] += 1
                e_cache[t] = scores_exp_mask(t[1], t[2], qt_t, mask_t)

            # When the SBUF e-pool is full but ACT is starving (qc0), emit
            # far-ahead tasks anyway and round-trip the masked-exp tile
            # through DRAM. Spill + unspill ride the SAME DMA queue so the
            # queue's serial order enforces the RAW dependency on the slot.
            spilled = {}
            spill_n = [0]

            def spill_task(t, qt_t, mask_t):
                emitted.add(t)
                emit_seq[t] = seq[0]
                seq[0] += 1
                et = scores_exp_mask(t[1], t[2], qt_t, mask_t)
                s = spill_n[0]
                spill_n[0] += 1
                eng = nc.sync if s % 2 == 0 else nc.gpsimd
                eng.dma_start(esp[s * 128:(s + 1) * 128, :],
                              et[:].rearrange("p a b -> p (a b)"))
                spilled[t] = (s, eng)

            def unspill(t):
                s, eng = spilled.pop(t)
                et = ep.tile([128, 2, QCS], BF16, tag="e", name="e_u")
                eng.dma_start(et[:].rearrange("p a b -> p (a b)"),
                              esp[s * 128:(s + 1) * 128, :])
                e_cache[t] = et

            # the preamble already emitted (0, 0, 0..3); register them
            for _k in range(4):
                emitted.add((0, 0, _k))
                emit_seq[(0, 0, _k)] = seq[0]
                seq[0] += 1

            def pump(maxn=2):
                qc, j, kt = cursor["qc"], cursor["j"], cursor["kt"]
                n = 0
                scan = cursor["i"]
                while (scan < len(tasks) and scan < cursor["i"] + LOOKAHEAD
                       and n < maxn and len(e_cache) < CAP):
                    t = tasks[scan]
                    if t in emitted:
                        scan += 1
                        if scan == cursor["i"] + 1:
                            cursor["i"] = scan
                        continue
                    tqc, tj, tkt = t
                    if tqc == qc:
                        if qc == 0 and j == 0 and kt < KREADY[tkt // 4]:
                            scan += 1
                            continue
                        emit_task(t, qt_cur, mask_cur)
                    elif tqc == qc + 1 and j >= 1:
                        # qproj(d) for qc+1 runs at (j1, kt 4+2d); mask
                        # halves arrive at (j1, kt0/kt2).
                        if j == 1 and (kt <= 4 + 2 * tj or qt_next is None):
                            break
                        emit_task(t, qt_next, mask_next)
                    else:
                        break
                    n += 1
                    if scan == cursor["i"]:
                        cursor["i"] = scan + 1
                    scan += 1
                if (n < maxn and spill_n[0] < NSPILL and qc == 0
                        and len(e_cache) >= CAP):
                    while (scan < len(tasks)
                           and scan < cursor["i"] + LOOKAHEAD and n < maxn
                           and spill_n[0] < NSPILL):
                        t = tasks[scan]
                        if t in emitted:
                            scan += 1
                            continue
                        tqc, tj, tkt = t
                        if tqc == qc:
                            if qc == 0 and j == 0 and kt < KREADY[tkt // 4]:
                                scan += 1
                                continue
                            spill_task(t, qt_cur, mask_cur)
                        elif tqc == qc + 1 and j >= 1:
                            if j == 1 and (kt <= 4 + 2 * tj
                                           or qt_next is None):
                                break
                            spill_task(t, qt_next, mask_next)
                        else:
                            break
                        n += 1
                        scan += 1
                # advance cursor past any contiguous emitted prefix
                while (cursor["i"] < len(tasks)
                       and tasks[cursor["i"]] in emitted):
                    cursor["i"] += 1

            for qc in range(QCN):
                ct_cur = [None] * 4
                for j in range(4):
                    pv = [psPV.tile([128, 4, 128], F32, tag=f"pv{hh}",
                                    name=f"pv{hh}")
                          for hh in range(2)]
                    j0 = qc == 0 and j == 0
                    lag = 2 if j0 else 0
                    nxt = 0     # next kt of this block to pv-consume
                    for kt in range(KT):
                        slot_seq = seq[0]
                        cursor["qc"], cursor["j"], cursor["kt"] = qc, j, kt
                        # --- interleaved projection / prefetch work ---
                        # outproj of the previous q chunk sits a few slots in
                        # so it never delays this block's first exp
                        if ct_prev is not None and kt in (4, 6):
                            outproj(qc - 1, 2 * j + (kt - 4) // 2, ct_prev)
                        if j0:
                            if kt in (1, 5, 9):
                                xk_next = load_xk(kt // 4 + 1)
                            if kt in (3, 7, 11):
                                for d in range(4):
                                    kproj(kt // 4 + 1, d, xk_next)
                                    pump(1)
                            if kt in (4, 8, 12):
                                xv_next = load_xv(kt // 4)
                            if kt >= 1:
                                vproj(kt - 1, xv_cur)
                                pump(1)
                            if kt in (4, 8, 12):
                                xv_cur = xv_next
                        if qc == 0 and j == 1 and kt == 0:
                            nc.sync.dma_start(
                                wo_full[:, 0:2, :], woT[0:256, :].rearrange(
                                    "(t p) c -> p t c", p=128))
                            nc.gpsimd.dma_start(
                                wo_full[:, 2:4, :], woT[256:DH, :].rearrange(
                                    "(t p) c -> p t c", p=128))
                        # qproj(qc+1): all four head-pair dims early in
                        # j1 so the cursor can run into the next chunk.
                        if j == 1 and kt == 0 and qc + 1 < QCN:
                            xq_next = load_xq(qc + 1)
                            mask_next = [load_mask_half(qc + 1, 0), None]
                        if j == 1 and kt == 2 and qc + 1 < QCN:
                            mask_next[1] = load_mask_half(qc + 1, 1)
                        if j == 1 and kt in (4, 6, 8, 10) and qc + 1 < QCN:
                            d = (kt - 4) // 2
                            if d == 0:
                                qt_next = qtp.tile([128, 4, QCS], BF16,
                                                   tag="qt", name="qt_t")
                            qproj(d, qt_next, xq_next)
                        if kt == 1 and pend_tr is not None:
                            transpose_ct(*pend_tr)
                            pend_tr = None
                        # --- attention slot ---
                        # issue unspills ~10 consumption slots ahead so the
                        # DRAM round-trip completes before pv needs the tile
                        if spilled and len(e_cache) < CAP + 2:
                            base = (qc * 4 + j) * KT + nxt
                            for t_u in tasks[base:base + 10]:
                                if t_u in spilled:
                                    unspill(t_u)
                                    break
                        # keep emission at least at the slot pointer
                        if nxt <= kt and (qc, j, kt) not in emitted:
                            emit_task((qc, j, kt), qt_cur, mask_cur)
                        # consume every seasoned cached tile in kt order so
                        # the e-pool frees up during long proj bursts. In j0
                        # consumption must trail the vproj stream: vproj(st)
                        # is only emitted at slot st+1.
                        while nxt < KT:
                            if j0 and nxt >= kt:
                                break
                            k2 = (qc, j, nxt)
                            if k2 not in e_cache:
                                break
                            if not (emit_seq[k2] < slot_seq
                                    or nxt <= kt - lag):
                                break
                            pv_accum(j, nxt, e_cache.pop(k2), pv)
                            nxt += 1
                            pump(1)
                        pump(3 if j0 else 2)
                    if j0:
                        vproj(KT - 1, xv_cur)
                    while nxt < KT:
                        k2 = (qc, j, nxt)
                        if k2 not in emitted:
                            emit_task(k2, qt_cur, mask_cur)
                        elif k2 in spilled:
                            unspill(k2)
                        pv_accum(j, nxt, e_cache.pop(k2), pv)
                        nxt += 1
                    if not j0:
                        if qc == QCN - 1 and j == 3:
                            # tail wave 1: jj 0-2 outproj partials keep the
                            # PE in its warm p-state through the last
                            # normalize + transpose and shorten the tail.
                            # psC stays free for the final transpose psum.
                            tail_sps = [psS.tile([128, 2, QCS], F32,
                                                 tag="sps", name="tail_sps")
                                        for _ in range(2)]
                            tail_psc = [psC.tile([128, QCS], F32,
                                                 tag="ps", name="tail_poc")
                                        for _ in range(2)]
                            for oc in range(6):
                                po = (tail_sps[oc // 2][:, oc % 2, :]
                                      if oc < 4 else tail_psc[oc - 4][:])
                                tail_po[oc] = po
                                for jj in range(3):
                                    nc.tensor.matmul(
                                        po,
                                        wo_full[:, jj, oc * 128:(oc + 1) * 128],
                                        ct_cur[jj][:],
                                        start=(jj == 0), stop=False)
                    is_tail = qc == QCN - 1 and j == 3
                    nrm_t = normalize_transpose(j, pv, ct_cur, tail=is_tail)
                    if is_tail:
                        transpose_ct_tail(nrm_t, ct_cur, j)
                    else:
                        pend_tr = (nrm_t, ct_cur, j)
                ct_prev = ct_cur
                if qc + 1 < QCN:
                    mask_cur = mask_next
                    qt_cur = qt_next
            # tail wave 2: close the six partial projections with their
            # jj=3 term, then run the last two o-chunks on recycled PSUM
            # banks; output copies alternate DVE / ACT.
            for oc in range(6):
                po = tail_po[oc]
                nc.tensor.matmul(
                    po, wo_full[:, 3, oc * 128:(oc + 1) * 128],
                    ct_prev[3][:], start=False, stop=True)
                o_t = op_.tile([128, QCS], BF16, tag="o", name="o_t")
                if oc % 2 == 0:
                    nc.vector.tensor_copy(o_t[:], po)
                else:
                    nc.scalar.activation(o_t[:], po, IDENT)
                (nc.sync if oc % 2 == 0 else nc.gpsimd).dma_start(
                    outT[oc * 128:(oc + 1) * 128,
                         (QCN - 1) * QCS:QCN * QCS], o_t[:])
            po6 = psPV.tile([128, 4, 128], F32, tag="pv1",
                            name="po6")[:].rearrange("p a b -> p (a b)")
            po7t = psS.tile([128, 2, QCS], F32, tag="sps", name="po7")
            for oc, po in ((6, po6), (7, po7t[:, 0, :])):
                for jj in range(4):
                    nc.tensor.matmul(
                        po, wo_full[:, jj, oc * 128:(oc + 1) * 128],
                        ct_prev[jj][:], start=(jj == 0), stop=(jj == 3))
                o_t = op_.tile([128, QCS], BF16, tag="o", name="o_t")
                if oc % 2 == 0:
                    nc.vector.tensor_copy(o_t[:], po)
                else:
                    nc.scalar.activation(o_t[:], po, IDENT)
                (nc.sync if oc % 2 == 0 else nc.gpsimd).dma_start(
                    outT[oc * 128:(oc + 1) * 128,
                         (QCN - 1) * QCS:QCN * QCS], o_t[:])

    nc.finalize()
    return nc


_NC_CACHE = None


def _get_nc():
    global _NC_CACHE
    if _NC_CACHE is None:
        _NC_CACHE = build_nc()
    return _NC_CACHE


_OUT_CONST = {}


def shard_inputs(query, key, value, mask, wq, bq, wk, bk, wv, bv, wo, bo):
    """Build the per-core input maps (host-side shard prep)."""
    import ml_dtypes

    query = np.asarray(query, np.float32)
    key = np.asarray(key, np.float32)
    value = np.asarray(value, np.float32)
    mask = np.asarray(mask)
    wq = np.asarray(wq, np.float32); bq = np.asarray(bq, np.float32)
    wk = np.asarray(wk, np.float32); bk = np.asarray(bk, np.float32)
    wv = np.asarray(wv, np.float32); bv = np.asarray(bv, np.float32)
    wo = np.asarray(wo, np.float32); bo = np.asarray(bo, np.float32)

    # softmax weights sum to 1, so the V bias contributes exactly wo @ bv to
    # every output row; fold it (and bo) into one host-side constant.
    _OUT_CONST["c"] = (bo + wo @ bv).astype(np.float32)

    in_maps = []
    maskT_b = [np.ascontiguousarray(mask[b].T).astype(ml_dtypes.bfloat16)
               for b in range(B)]
    xT = {}
    for b in range(B):
        xT[b] = (
            np.ascontiguousarray(query[b].T).astype(ml_dtypes.bfloat16),
            np.ascontiguousarray(key[b].T).astype(ml_dtypes.bfloat16),
            np.ascontiguousarray(value[b].T).astype(ml_dtypes.bfloat16),
        )
    for c in range(NCORES):
        b, hg = divmod(c, HG)
        sl = slice(hg * DH, (hg + 1) * DH)
        wo_block = wo[:, sl]                       # [1024, 512]
        in_maps.append({
            "esp": np.zeros((NSPILL * 128, 2 * (S // 4)),
                            ml_dtypes.bfloat16),
            "xqT": xT[b][0],
            "xkT": xT[b][1],
            "xvT": xT[b][2],
            "maskT": maskT_b[b],
            "wqT": np.ascontiguousarray(wq[sl].T).astype(ml_dtypes.bfloat16),
            "wkT": np.ascontiguousarray(wk[sl].T).astype(ml_dtypes.bfloat16),
            "wvT": np.ascontiguousarray(wv[sl].T).astype(ml_dtypes.bfloat16),
            "woT": np.ascontiguousarray(wo_block.T).astype(ml_dtypes.bfloat16),
            "bq2": np.ascontiguousarray((bq[sl] / 8.0).reshape(DH // 128, 128).T),
            "bk2": np.ascontiguousarray(bk[sl].reshape(DH // 128, 128).T),
        })
    return in_maps


def combine_outputs(results):
    """results: list of per-core {"outT": [1024, 2048]} -> full [B, S, D]."""
    out = np.empty((B, S, D), np.float32)
    c = _OUT_CONST["c"]
    for b in range(B):
        acc = (results[2 * b]["outT"].astype(np.float32)
               + results[2 * b + 1]["outT"].astype(np.float32))
        out[b] = acc.T + c
    return out


def kernel(**inputs):
    from concourse.bass_utils import run_bass_kernel_spmd

    nc = _get_nc()
    in_maps = shard_inputs(**inputs)
    res = run_bass_kernel_spmd(nc, in_maps, list(range(NCORES)))
    return combine_outputs(res.results)

